# revision 1
# baseline (speedup 1.0000x reference)
"""MixHop GNN kernel for one TRN2 chip (8 NeuronCores), Bass/Tile.

Math (matches the reference exactly):
    row/col = edge_index with self loops appended
    deg[t]  = #edges with col==t            (host: integer bincount)
    dinv    = 1/sqrt(deg)                   (device: sqrt + reciprocal)
    prop(h) = D^-1/2 A D^-1/2 h  with A[c,r] = 1 per edge
            -> z = dinv*h (per row); y = A z (gather+segment-sum); out = dinv*y
    h1 = prop(x); h2 = prop(h1)
    out = relu(concat(x@W0+b0, h1@W1+b1, h2@W2+b2)) @ Wout + bout

Sharding: core c owns target nodes [c*N/8, (c+1)*N/8). Edges (self loops
excluded -- those are added locally with a DVE add since z[t] is resident)
are bucketed by target into windows of 128 consecutive targets. Window
edges are split by source (< 32768 vs >=, the int16 limit of dma_gather),
each part padded to blocks of 128 (uniform across cores -> one SPMD
program). Per window: dma_gather pulls the source rows of z (512B each),
a one-hot selection matrix S (tl==iota) and a PE matmul S.T @ G accumulate
the segment-sum in PSUM. z vectors are exchanged between hops with an
AllGather.
"""
import numpy as np
import ml_dtypes

N = 50000
F = 128
NCORE = 8
NPC = N // NCORE          # 6250 nodes per core
WIN = 128                 # targets per window
NWIN = (NPC + WIN - 1) // WIN   # 49 (48 full + 1 partial of 106)
PER_HOP = 64
OUT = 64
SPLIT = 32768             # int16 index limit for dma_gather tables
MAXBLK = 8                # max 1024 idxs per dma_gather call
PAD_TL = 300.0            # dummy-edge tl: matches no iota value -> zero S row


def _chunks(nb):
    out = []
    while nb > 0:
        c = min(nb, MAXBLK)
        out.append(c)
        nb -= c
    return out


def _preprocess(edge_index):
    """Bucket edges by (core, target-window, source-half); pad uniformly.

    Returns (NBL, NBH, per_core list of dicts with idx16, tl_t, deg_t).
    """
    row = np.asarray(edge_index[0], dtype=np.int64)
    col = np.asarray(edge_index[1], dtype=np.int64)
    deg = (np.bincount(col, minlength=N) + 1).astype(np.float32)  # + self loop

    cores = []
    maxlo = maxhi = 0
    for c in range(NCORE):
        lo, hi = c * NPC, (c + 1) * NPC
        sel = (col >= lo) & (col < hi)
        r = row[sel]
        t = col[sel] - lo
        order = np.argsort(t, kind="stable")
        r, t = r[order], t[order]
        is_lo = r < SPLIT
        parts = []
        for w in range(NWIN):
            wsel = (t // WIN) == w
            rl = r[wsel & is_lo]
            tl_l = (t[wsel & is_lo] % WIN).astype(np.float32)
            rh = r[wsel & ~is_lo] - SPLIT
            tl_h = (t[wsel & ~is_lo] % WIN).astype(np.float32)
            maxlo = max(maxlo, len(rl))
            maxhi = max(maxhi, len(rh))
            parts.append((rl, tl_l, rh, tl_h))
        cores.append(parts)

    # per-window-index block counts (max over the 8 cores) -> less padding
    NBLs = tuple(int((max(len(cores[c][w][0]) for c in range(NCORE)) + 127)
                     // 128) for w in range(NWIN))
    NBHs = tuple(int((max(len(cores[c][w][2]) for c in range(NCORE)) + 127)
                     // 128) for w in range(NWIN))
    idx_cols = [(nl + nh) * 8 for nl, nh in zip(NBLs, NBHs)]
    IDXOFF = np.concatenate([[0], np.cumsum(idx_cols)]).astype(int)
    TLOFF = np.concatenate(
        [[0], np.cumsum([nl + nh for nl, nh in zip(NBLs, NBHs)])]).astype(int)
    out = []
    for parts in cores:
        idx16 = np.zeros((128, int(IDXOFF[-1])), dtype=np.int16)
        tl_t = np.full((128, int(TLOFF[-1])), PAD_TL, dtype=np.float32)
        for w, (rl, tl_l, rh, tl_h) in enumerate(parts):
            nbl_w, nbh_w = NBLs[w], NBHs[w]
            nbt_w = nbl_w + nbh_w
            lo_chunks, hi_chunks = _chunks(nbl_w), _chunks(nbh_w)
            ilo = np.zeros(nbl_w * 128, dtype=np.int16)
            ilo[:len(rl)] = rl
            ihi = np.zeros(nbh_w * 128, dtype=np.int16)
            ihi[:len(rh)] = rh
            # tl stream: [lo blocks..., hi blocks...], PAD_TL in padding
            tw = np.full(nbt_w * 128, PAD_TL, dtype=np.float32)
            tw[:len(tl_l)] = tl_l
            tw[nbl_w * 128:nbl_w * 128 + len(tl_h)] = tl_h
            tl_t[:, TLOFF[w]:TLOFF[w + 1]] = tw.reshape(nbt_w, 128).T
            # idx stream: per call, [16-wrap then replicate x8 partitions]
            cols = []
            off = 0
            for cb in lo_chunks:
                a = ilo[off:off + cb * 128]
                cols.append(np.tile(a.reshape(-1, 16).T, (8, 1)))
                off += cb * 128
            off = 0
            for cb in hi_chunks:
                a = ihi[off:off + cb * 128]
                cols.append(np.tile(a.reshape(-1, 16).T, (8, 1)))
                off += cb * 128
            if cols:
                idx16[:, IDXOFF[w]:IDXOFF[w + 1]] = np.concatenate(
                    cols, axis=1)
        out.append({"idx16": np.ascontiguousarray(idx16),
                    "tl_t": np.ascontiguousarray(tl_t)})

    for c in range(NCORE):
        lo = c * NPC
        dpad = np.ones(NWIN * WIN, dtype=np.float32)
        dpad[:NPC] = deg[lo:lo + NPC]
        out[c]["deg_t"] = np.ascontiguousarray(
            dpad.reshape(NWIN, WIN).T)
    return NBLs, NBHs, out


def _build(NBLs, NBHs):
    import concourse.bass as bass  # noqa: F401
    import concourse.bacc as bacc
    import concourse.tile as tile
    import concourse.mybir as mybir
    from concourse.masks import make_identity

    dt = mybir.dt
    f32 = dt.float32
    AF = mybir.ActivationFunctionType
    ALU = mybir.AluOpType
    NBTs = [nl + nh for nl, nh in zip(NBLs, NBHs)]
    NBTMAX = max(NBTs)
    IDXOFF = np.concatenate(
        [[0], np.cumsum([nbt * 8 for nbt in NBTs])]).astype(int)
    TLOFF = np.concatenate([[0], np.cumsum(NBTs)]).astype(int)
    NFULL = (NWIN - 1) * WIN                 # 6144 rows in full windows
    NLAST = NPC - NFULL                      # 106 rows in the partial window

    nc = bacc.Bacc("TRN2", target_bir_lowering=False, debug=False,
                   num_devices=NCORE)

    x_own = nc.dram_tensor("x_own", [NPC, F], f32, kind="ExternalInput")
    bf16 = dt.bfloat16
    idx_in = nc.dram_tensor("idx16", [128, int(IDXOFF[-1])], dt.int16,
                            kind="ExternalInput")
    tl_in = nc.dram_tensor("tl_t", [128, int(TLOFF[-1])], f32,
                           kind="ExternalInput")
    deg_in = nc.dram_tensor("deg_t", [128, NWIN], f32, kind="ExternalInput")
    iot_in = nc.dram_tensor("iot", [128, 128], bf16, kind="ExternalInput")
    w0_in = nc.dram_tensor("w0", [F, PER_HOP], f32, kind="ExternalInput")
    w1_in = nc.dram_tensor("w1", [F, PER_HOP], f32, kind="ExternalInput")
    w2_in = nc.dram_tensor("w2", [F, PER_HOP], f32, kind="ExternalInput")
    wo_in = nc.dram_tensor("wout", [3 * PER_HOP, OUT], f32,
                           kind="ExternalInput")
    b0_in = nc.dram_tensor("b0", [PER_HOP, 1], f32, kind="ExternalInput")
    b1_in = nc.dram_tensor("b1", [PER_HOP, 1], f32, kind="ExternalInput")
    b2_in = nc.dram_tensor("b2", [PER_HOP, 1], f32, kind="ExternalInput")
    bo_in = nc.dram_tensor("bout", [OUT, 1], f32, kind="ExternalInput")
    out_t = nc.dram_tensor("out_t", [OUT, NPC], f32, kind="ExternalOutput")

    z0b = nc.dram_tensor("z0b", [NPC, F], bf16)
    z1b = nc.dram_tensor("z1b", [NPC, F], bf16)
    z0f = nc.dram_tensor("z0f", [N, F], bf16, addr_space="Shared")
    z1f = nc.dram_tensor("z1f", [N, F], bf16, addr_space="Shared")

    def ts(w):
        return slice(w * WIN, (w + 1) * WIN)

    with tile.TileContext(nc) as tc:
        with (
            tc.tile_pool(name="persist", bufs=1) as pp,
            tc.tile_pool(name="gbuf", bufs=4) as gp,
            tc.tile_pool(name="work", bufs=4) as wp,
            tc.tile_pool(name="psum_y", bufs=2, space="PSUM") as psy,
            tc.tile_pool(name="psum_t", bufs=2, space="PSUM") as pst,
            tc.tile_pool(name="psum_d", bufs=2, space="PSUM") as psd,
        ):
            # ---- persistent loads ----
            idx_sb = pp.tile([128, int(IDXOFF[-1])], dt.int16)
            nc.sync.dma_start(out=idx_sb[:], in_=idx_in[:])
            tl_sb = pp.tile([128, int(TLOFF[-1])], f32)
            nc.sync.dma_start(out=tl_sb[:], in_=tl_in[:])
            iot_sb = pp.tile([128, 128], bf16)
            nc.sync.dma_start(out=iot_sb[:], in_=iot_in[:])
            deg_sb = pp.tile([128, NWIN], f32)
            nc.sync.dma_start(out=deg_sb[:], in_=deg_in[:])
            w0_sb = pp.tile([F, PER_HOP], f32)
            nc.sync.dma_start(out=w0_sb[:], in_=w0_in[:])
            w1_sb = pp.tile([F, PER_HOP], f32)
            nc.sync.dma_start(out=w1_sb[:], in_=w1_in[:])
            w2_sb = pp.tile([F, PER_HOP], f32)
            nc.sync.dma_start(out=w2_sb[:], in_=w2_in[:])
            wo_sb = []
            for k in range(3):
                t = pp.tile([PER_HOP, OUT], f32, tag=f"wo{k}")
                nc.sync.dma_start(
                    out=t[:], in_=wo_in.ap()[k * PER_HOP:(k + 1) * PER_HOP, :])
                wo_sb.append(t)
            b_sb = []
            for k, bin_ in enumerate((b0_in, b1_in, b2_in)):
                t = pp.tile([PER_HOP, 1], f32, tag=f"b{k}")
                nc.sync.dma_start(out=t[:], in_=bin_[:])
                b_sb.append(t)
            bo_sb = pp.tile([OUT, 1], f32)
            nc.sync.dma_start(out=bo_sb[:], in_=bo_in[:])
            ident = pp.tile([128, 128], f32)
            make_identity(nc, ident[:])

            # dinv = 1/sqrt(deg); dinv2 = dinv^2  (both [128, NWIN])
            sq = pp.tile([128, NWIN], f32)
            nc.scalar.activation(out=sq[:], in_=deg_sb[:], func=AF.Sqrt)
            dinv = pp.tile([128, NWIN], f32)
            nc.vector.reciprocal(out=dinv[:], in_=sq[:])
            dinv2 = pp.tile([128, NWIN], f32)
            nc.vector.tensor_tensor(out=dinv2[:], in0=dinv[:], in1=dinv[:],
                                    op=ALU.mult)

            # ---- load x (window-major: [p, w*128+f] = x[w*128+p, f]) ----
            x_sb = pp.tile([128, NWIN * WIN], f32)
            nc.vector.memset(x_sb[:, (NWIN - 1) * WIN:], 0.0)
            nc.sync.dma_start(
                out=x_sb[:].rearrange("p (w f) -> p w f", f=F)[:, 0:NWIN - 1, :],
                in_=x_own.ap()[0:NFULL, :].rearrange("(w p) f -> p w f", p=128),
            )
            nc.sync.dma_start(
                out=x_sb[0:NLAST, (NWIN - 1) * WIN:],
                in_=x_own.ap()[NFULL:NPC, :],
            )

            z_stage = pp.tile([128, NWIN * WIN], bf16)


            def stage_to_bounce(zbounce):
                nc.sync.dma_start(
                    out=zbounce.ap()[0:NFULL, :].rearrange(
                        "(w p) f -> p w f", p=128),
                    in_=z_stage[:].rearrange(
                        "p (w f) -> p w f", f=F)[:, 0:NWIN - 1, :],
                )
                nc.sync.dma_start(
                    out=zbounce.ap()[NFULL:NPC, :],
                    in_=z_stage[0:NLAST, (NWIN - 1) * WIN:],
                )

            # z0 = dinv * x
            for w in range(NWIN):
                nc.vector.tensor_scalar_mul(
                    out=z_stage[:, ts(w)], in0=x_sb[:, ts(w)],
                    scalar1=dinv[:, w:w + 1])
            stage_to_bounce(z0b)
            nc.gpsimd.collective_compute(
                "AllGather", ALU.bypass,
                replica_groups=[list(range(NCORE))],
                ins=[z0b[:]], outs=[z0f[:]])

            # ---- one propagation sweep ----
            # h_out = dinv * (sum_edges z[src] + z_self); z_out = dinv^2 * (...)
            def prop(zf, h_out, z_out):
                for w in range(NWIN):
                    NBT = NBTs[w]
                    g = gp.tile([128, NBTMAX * F], bf16, tag="g")
                    icol = int(IDXOFF[w])
                    blk = 0
                    for cb in _chunks(NBLs[w]):
                        nc.gpsimd.dma_gather(
                            out_ap=g[:, blk * F:(blk + cb) * F].rearrange(
                                "p (b f) -> p b f", f=F),
                            in_ap=zf.ap()[0:SPLIT, :],
                            idxs_ap=idx_sb[:, icol:icol + cb * 8],
                            num_idxs=cb * 128, num_idxs_reg=cb * 128,
                            elem_size=F)
                        icol += cb * 8
                        blk += cb
                    for cb in _chunks(NBHs[w]):
                        nc.gpsimd.dma_gather(
                            out_ap=g[:, blk * F:(blk + cb) * F].rearrange(
                                "p (b f) -> p b f", f=F),
                            in_ap=zf.ap()[SPLIT:N, :],
                            idxs_ap=idx_sb[:, icol:icol + cb * 8],
                            num_idxs=cb * 128, num_idxs_reg=cb * 128,
                            elem_size=F)
                        icol += cb * 8
                        blk += cb
                    ps = psy.tile([128, F], f32)
                    for j in range(NBT):
                        col = int(TLOFF[w]) + j
                        s = wp.tile([128, 128], bf16, tag="s")
                        nc.vector.tensor_scalar(
                            out=s[:], in0=iot_sb[:],
                            scalar1=tl_sb[:, col:col + 1], scalar2=None,
                            op0=ALU.is_equal)
                        nc.tensor.matmul(
                            out=ps[:], lhsT=s[:], rhs=g[:, j * F:(j + 1) * F],
                            start=(j == 0), stop=(j == NBT - 1))
                    # self loop: y += z[own window]
                    ya = wp.tile([128, F], f32, tag="ya")
                    nc.vector.tensor_tensor(
                        out=ya[:], in0=ps[:], in1=z_stage[:, ts(w)],
                        op=ALU.add)
                    if z_out is not None:
                        nc.vector.tensor_scalar_mul(
                            out=z_out[:, ts(w)], in0=ya[:],
                            scalar1=dinv2[:, w:w + 1])
                    nc.vector.tensor_scalar_mul(
                        out=h_out[:, ts(w)], in0=ya[:],
                        scalar1=dinv[:, w:w + 1])

            h1_sb = pp.tile([128, NWIN * WIN], f32)
            prop(z0f, h1_sb, z_stage)   # z_stage: z0 -> z1 (read self first)
            stage_to_bounce(z1b)
            nc.gpsimd.collective_compute(
                "AllGather", ALU.bypass,
                replica_groups=[list(range(NCORE))],
                ins=[z1b[:]], outs=[z1f[:]])

            h2_sb = pp.tile([128, NWIN * WIN], f32)
            prop(z1f, h2_sb, None)

            # ---- dense MixHop head (computes out.T tiles [64, 128]) ----
            out_stage = pp.tile([OUT, NWIN * WIN], f32)
            hops = ((x_sb, w0_sb), (h1_sb, w1_sb), (h2_sb, w2_sb))
            for w in range(NWIN):
                relus = []
                for k, (h_sb, wk_sb) in enumerate(hops):
                    tp = pst.tile([128, 128], f32, tag="tp")
                    nc.tensor.transpose(out=tp[:], in_=h_sb[:, ts(w)],
                                        identity=ident[:])
                    hT = wp.tile([128, 128], f32, tag="hT")
                    nc.vector.tensor_copy(out=hT[:], in_=tp[:])
                    cps = psd.tile([PER_HOP, 128], f32, tag="cps")
                    nc.tensor.matmul(out=cps[:], lhsT=wk_sb[:], rhs=hT[:],
                                     start=True, stop=True)
                    rk = wp.tile([PER_HOP, 128], f32, tag=f"r{k}")
                    nc.scalar.activation(out=rk[:], in_=cps[:], func=AF.Relu,
                                         bias=b_sb[k][:])
                    relus.append(rk)
                ops = psd.tile([OUT, 128], f32, tag="ops")
                for k in range(3):
                    nc.tensor.matmul(out=ops[:], lhsT=wo_sb[k][:],
                                     rhs=relus[k][:],
                                     start=(k == 0), stop=(k == 2))
                nc.scalar.activation(out=out_stage[:, ts(w)], in_=ops[:],
                                     func=AF.Identity, bias=bo_sb[:])
            nc.sync.dma_start(out=out_t[:], in_=out_stage[:, 0:NPC])

    nc.compile()
    return nc


_CACHE = {}


def _get_nc(NBLs, NBHs):
    key = (tuple(NBLs), tuple(NBHs))
    if key not in _CACHE:
        _CACHE[key] = _build(NBLs, NBHs)
    return _CACHE[key]


def make_in_maps(x, pc, W0, b0, W1, b1, W2, b2, Wout, bout):
    iot = np.broadcast_to(
        np.arange(128, dtype=np.float32), (128, 128)).astype(ml_dtypes.bfloat16)
    common = {
        "iot": iot,
        "w0": np.asarray(W0, dtype=np.float32),
        "w1": np.asarray(W1, dtype=np.float32),
        "w2": np.asarray(W2, dtype=np.float32),
        "wout": np.asarray(Wout, dtype=np.float32),
        "b0": np.asarray(b0, dtype=np.float32).reshape(PER_HOP, 1),
        "b1": np.asarray(b1, dtype=np.float32).reshape(PER_HOP, 1),
        "b2": np.asarray(b2, dtype=np.float32).reshape(PER_HOP, 1),
        "bout": np.asarray(bout, dtype=np.float32).reshape(OUT, 1),
    }
    x = np.ascontiguousarray(np.asarray(x, dtype=np.float32))
    in_maps = []
    for c in range(NCORE):
        m = dict(common)
        m.update(pc[c])
        m["x_own"] = np.ascontiguousarray(x[c * NPC:(c + 1) * NPC])
        in_maps.append(m)
    return in_maps


def kernel(x, edge_index, W0, b0, W1, b1, W2, b2, Wout, bout):
    from concourse.bass_utils import run_bass_kernel_spmd

    NBL, NBH, pc = _preprocess(np.asarray(edge_index))
    nc = _get_nc(NBL, NBH)
    in_maps = make_in_maps(x, pc, W0, b0, W1, b1, W2, b2, Wout, bout)
    res = run_bass_kernel_spmd(nc, in_maps, core_ids=list(range(NCORE)))
    out = np.empty((N, OUT), dtype=np.float32)
    for c in range(NCORE):
        out[c * NPC:(c + 1) * NPC] = res.results[c]["out_t"].T
    return out

